# revision 47
# baseline (speedup 1.0000x reference)
"""MoE routed-expert kernel for Trainium2 (8 NeuronCores, SPMD).

Problem: N=16384 tokens, D=768, H=768, C=2, E=20 experts.
  y[n] = relu(x[n] @ W1[e] + b1[e]) @ W2[e] + b2[e],  e = component_idx[n]

Strategy
--------
Host side (numpy): sort tokens by expert, split the token groups into
8*n_slots fragments (splitting the largest until the count matches), deal
the size-sorted fragments into 8 cores x n_slots "expert slots" with a
uniform per-slot capacity = max fragment size in that slot. Every core then
runs the SAME static program (SPMD); which expert a slot holds is purely a
matter of which weights/tokens the host stages into that core's input
buffers. Padding waste is ~2%.

Device side (Bass/Tile, per core): for each slot, load that expert's
W1 [768,768] (+ b1, W2, b2) and the slot's token block x^T with D on
partitions and tokens on the free dim. For each chunk of 256..512 tokens:
  layer1: 6x6 accumulating fp16 matmuls (full PE speed)
  relu+bias fused on ScalarE (PSUM fp32 -> SBUF fp16)
  layer2: 6 accumulating matmuls into a [2, T] PSUM, bias via VectorE

Pipeline engineering (the ~20% between naive and roofline):
- Input DMA issue (DIRECT2D, ~650ns each on the issuing sequencer) is
  spread across the four non-Tensor engines so the whole fill is in
  flight within ~1.5us of body start, instead of 30us serialized on Sync.
- Slot 0 streams fine-grained per-d-tile slabs (first matmul starts when
  the first ~0.4MB lands); slots 1-2 use one coarse DMA for w1 and one
  for x (fewer descriptors, >9KB contiguous lines).
- PE warm-up: 6 independent small matmuls on rotating PSUM banks right
  at body start flip the HAM clock gate to 2.4 GHz with no WAW chain.
- A dummy activation preloads the ScalarE activation table during fill.
- The serial relu chain of the last two chunks is split Scalar/Vector.
"""

import math

import numpy as np

import concourse.bass as bass  # noqa: F401
import concourse.mybir as mybir
from concourse import bacc
from concourse.bass_utils import run_bass_kernel_spmd
from concourse.tile import TileContext

F32 = mybir.dt.float32
F16 = mybir.dt.float16
MM_DT = F16
MM_NP = np.float16

N_CORES = 8
N_SLOTS = 3
D = 768
H = 768
C = 2
DT = D // 128  # 6 d-tiles
HT = H // 128  # 6 h-tiles
MAX_CHUNK = 512  # one PSUM bank holds 512 fp32 -> matmul free dim cap
MIN_CAP = 256  # keep chunks >=256 so per-matmul overhead stays amortized

RELU = mybir.ActivationFunctionType.Relu
ADD = mybir.AluOpType.add
MAX = mybir.AluOpType.max
MULT = mybir.AluOpType.mult


def _round_cap(cap: int) -> int:
    """Round capacity up so it splits into equal, even chunks of 256..512."""
    cap = max(cap, MIN_CAP)
    n = max(1, math.ceil(cap / MAX_CHUNK))
    return 2 * n * math.ceil(cap / (2 * n))


def _chunk_sizes(cap: int, tail_split: bool = False) -> list[int]:
    n = max(1, math.ceil(cap / MAX_CHUNK))
    assert cap % n == 0 and (cap // n) % 2 == 0, cap
    sizes = [cap // n] * n
    if tail_split and sizes[-1] > 256:
        # shorten the serial relu->L2->add->DMA chain after the last
        # layer-1 matmul of the program
        sizes[-1:] = [sizes[-1] - 128, 128]
    return sizes


def _plan_packing(counts: np.ndarray):
    """Return (caps, assign): per-slot capacities and
    assign[s][c] = (expert, start_within_group, length)."""
    frags = [(int(e), 0, int(c)) for e, c in enumerate(counts) if c > 0]
    target = N_CORES * N_SLOTS
    assert len(frags) <= target, (
        f"{len(frags)} non-empty experts exceed {target} slots; raise N_SLOTS"
    )
    while len(frags) < target:
        frags.sort(key=lambda f: -f[2])
        e, st, ln = frags[0]
        if ln < 2:
            frags.append((e, st, 0))
            continue
        h1 = ln // 2
        frags[0] = (e, st, ln - h1)
        frags.append((e, st + (ln - h1), h1))
    frags.sort(key=lambda f: -f[2])
    caps, assign = [], []
    for s in range(N_SLOTS):
        group = frags[s * N_CORES : (s + 1) * N_CORES]
        caps.append(_round_cap(max(f[2] for f in group)))
        assign.append(group)
    return caps, assign


_PROGRAM_CACHE: dict = {}


def _build_program(caps: tuple):
    if caps in _PROGRAM_CACHE:
        return _PROGRAM_CACHE[caps]

    R = sum(caps)
    nc = bacc.Bacc(
        "TRN2", target_bir_lowering=False, debug=False, num_devices=N_CORES
    )
    # x: per-slot blocks, dt-major inside a slot; per-partition lines of
    # DT*cap*2 bytes are contiguous so coarse DMAs get big descriptors.
    x = nc.dram_tensor("x", [128, DT * R], MM_DT, kind="ExternalInput")
    w1 = nc.dram_tensor("w1", [N_SLOTS, 128, DT * H], MM_DT, kind="ExternalInput")
    # one aux tensor: b1 ([128, S*HT]), b2 (cols S*HT+s, partitions 0-1),
    # and b2/128 broadcast down all partitions (cols S*HT+S+s*C+c) for the
    # DVE layer-2 path (the ones-matmul partition sum multiplies by 128).
    B2C = N_SLOTS * HT
    B2D = N_SLOTS * HT + N_SLOTS
    W2F = B2D + N_SLOTS * C  # fp32 copy of w2 (DVE scalars must be fp32)
    b1a = nc.dram_tensor(
        "b1a", [128, W2F + N_SLOTS * HT * C], F32, kind="ExternalInput"
    )
    b2a = nc.dram_tensor("b2a", [C, N_SLOTS], F32, kind="ExternalInput")
    w2a = nc.dram_tensor(
        "w2a", [128, N_SLOTS * HT * C], MM_DT, kind="ExternalInput"
    )
    y = nc.dram_tensor("y", [C, R], F32, kind="ExternalOutput")

    offs = [sum(caps[:s]) for s in range(N_SLOTS)]

    with TileContext(nc) as tc:
        with (
            tc.tile_pool(name="wpool", bufs=2) as wpool,
            tc.tile_pool(name="xpool", bufs=2) as xpool,
            tc.tile_pool(name="hpool", bufs=4) as hpool,
            tc.tile_pool(name="ypool", bufs=1) as ypool,
            tc.tile_pool(name="pspool", bufs=6, space="PSUM") as pspool,
            tc.tile_pool(name="pypool", bufs=1, space="PSUM") as pypool,
        ):
            # DVE layer-2 chunks: rows 0 and 32 hold y[0]/y[1] (the
            # ones-matmul writes 32 identical rows per c); the classic
            # tail writes rows 0-1 directly.
            y2_sb = ypool.tile([64, R], F32, name="y2_sb")

            # -- PE warm-up --------------------------------------------
            # ~3.2us of dummy matmuls bridge body-start to first-data
            # (~10us): the PE HAM clock-gate sees sustained activity and
            # un-throttles to 2.4GHz right as real matmuls begin. 3
            # accumulating matmuls per PSUM tile avoid WAW stalls.
            # memsets go on Vector so GpSimd's x-slab DMA issues start
            # immediately.
            wu = ypool.tile([128, 256], MM_DT, name="wu")
            nc.vector.memset(wu[:, :], 0.0)
            ones32 = ypool.tile([128, 32], MM_DT, name="ones32")
            nc.vector.memset(ones32[:, :], 1.0)
            for i in range(3):
                wu_ps = pspool.tile(
                    [128, 256], F32, name=f"wu_ps{i}", tag="psh"
                )
                for j in range(3):
                    nc.tensor.matmul(
                        wu_ps, wu[:, :128], wu[:, :],
                        start=(j == 0), stop=(j == 2),
                    )

            # -- payload fills -----------------------------------------
            # DMA-queue landing order tracks descriptor enqueue (= issue)
            # order, so the three DMA-capable sequencers (Sync/GpSimd/
            # Scalar) issue in strict need order: tiny aux first, slot0
            # d0..d5, slot1 d0..d5, slot2 coarse. Slots 0-1 use fine
            # per-d-tile slabs (dt-major first chunk starts on slab 0).
            w1_sl = {0: [None] * DT, 1: [None] * DT}
            xs_sl = {0: [None] * DT, 1: [None] * DT}
            # slot 0's x arrives in per-chunk halves so each dt round
            # costs ~300KB of fill -- under the PE's ~1.06us/round pace.
            half0 = caps[0] // 2
            split0 = len(_chunk_sizes(caps[0])) == 2
            xh_sl = {0: [None] * DT, 1: [None] * DT}

            def w1_dma(eng, s, dt):
                t = wpool.tile([128, H], MM_DT, name=f"w1_d{dt}", tag=f"w1d{dt}")
                if s == 0 and dt == 0:
                    # split the very first slab: the (dt0,ht0) matmul only
                    # needs 33KB to start during the slow DMA ramp
                    eng.dma_start(out=t[:, :128], in_=w1[s, :, :128])
                    eng.dma_start(out=t[:, 128:], in_=w1[s, :, 128 : H])
                else:
                    eng.dma_start(out=t, in_=w1[s, :, dt * H : (dt + 1) * H])
                w1_sl[s][dt] = t

            def x_dma(eng, s, dt):
                t = xpool.tile(
                    [128, caps[s]], MM_DT, name=f"xs_d{dt}", tag=f"xsd{dt}"
                )
                eng.dma_start(
                    out=t,
                    in_=x[
                        :,
                        DT * offs[s] + dt * caps[s] : DT * offs[s]
                        + (dt + 1) * caps[s],
                    ],
                )
                xs_sl[s][dt] = t

            def x_dma_half(eng, dt, hf):
                t = xpool.tile(
                    [128, half0], MM_DT, name=f"xh{hf}_d{dt}", tag=f"xh{hf}d{dt}"
                )
                base = DT * offs[0] + dt * caps[0] + hf * half0
                eng.dma_start(out=t, in_=x[:, base : base + half0])
                xh_sl[hf][dt] = t

            sy, gp, sc = nc.sync, nc.gpsimd, nc.scalar
            # aux first (tiny: ~28KB total, lands in ~0.1us)
            b1a_sb = ypool.tile(
                [128, W2F + N_SLOTS * HT * C], F32, name="b1a_sb"
            )
            sc.dma_start(out=b1a_sb, in_=b1a[:, :])
            w2a_sb = ypool.tile([128, N_SLOTS * HT * C], MM_DT, name="w2a_sb")
            sc.dma_start(out=w2a_sb, in_=w2a[:, :])
            b2a_sb = ypool.tile([C, N_SLOTS], F32, name="b2a_sb")
            sc.dma_start(out=b2a_sb, in_=b2a[:, :])
            # dummy activation: pull ACT_TABLE_LOAD into the fill window
            dmy = ypool.tile([128, 1], MM_DT, name="dmy")
            sc.activation(dmy, wu[:, :1], RELU, bias=b1a_sb[:, :1])
            # w1 slabs on sync, x slabs on gpsimd, slot-ordered; scalar
            # stays free for relus. Slot 2 coarse last.
            for s in (0, 1):
                for dt in range(DT):
                    w1_dma(sy, s, dt)
            w1_all = wpool.tile([128, DT * H], MM_DT, name="w1_all", tag="w1all")
            sy.dma_start(out=w1_all, in_=w1[2])
            if split0:
                for hf in (0, 1):
                    for dt in range(DT):
                        x_dma_half(gp, dt, hf)
            else:
                for dt in range(DT):
                    x_dma(gp, 0, dt)
            for dt in range(DT):
                x_dma(gp, 1, dt)
            xs_all = xpool.tile(
                [128, DT * caps[2]], MM_DT, name="xs_all", tag="xsall"
            )
            gp.dma_start(
                out=xs_all,
                in_=x[:, DT * offs[2] : DT * offs[2] + DT * caps[2]],
            )

            # -- compute -----------------------------------------------
            def w1_slice(s, dt, ht):
                if s < 2:
                    return w1_sl[s][dt][:, ht * 128 : (ht + 1) * 128]
                return w1_all[:, dt * H + ht * 128 : dt * H + (ht + 1) * 128]

            def x_slice(s, dt, lo, hi):
                if s == 0 and split0:
                    hf = 0 if hi <= half0 else 1
                    return xh_sl[hf][dt][:, lo - hf * half0 : hi - hf * half0]
                if s < 2:
                    return xs_sl[s][dt][:, lo:hi]
                return xs_all[:, dt * caps[s] + lo : dt * caps[s] + hi]

            pending = []
            for s in range(N_SLOTS):
                cap = caps[s]
                off = offs[s]

                chunk_list = _chunk_sizes(cap, tail_split=(s == N_SLOTS - 1))
                co = 0
                for ci, size in enumerate(chunk_list):
                    h_sb = hpool.tile([128, HT, size], MM_DT, name="h_sb", tag="h")

                    def relu(ht, ps):
                        b1_col = b1a_sb[:, s * HT + ht : s * HT + ht + 1]
                        # tail slot: halve the serial relu chain (short
                        # exit); DVE-L2 slots: vector is busy with the
                        # layer-2 accumulation, scalar takes most relus.
                        use_vec = (
                            ht % 2 == 1 if s == N_SLOTS - 1 else ht == 3
                        )
                        if use_vec:
                            # split the serial relu chain across engines
                            nc.vector.tensor_scalar(
                                h_sb[:, ht, :], ps, b1_col, 0.0, ADD, MAX
                            )
                        else:
                            nc.scalar.activation(
                                h_sb[:, ht, :], ps, RELU, bias=b1_col
                            )

                    if s == 0 or (s == 1 and ci == 0):
                        # dt-major: each dt round needs only that dt's two
                        # slabs -> PE starts while later slabs still stream
                        ps_list = [
                            pspool.tile(
                                [128, size], F32, name=f"ps_h{ht}", tag="psh"
                            )
                            for ht in range(HT)
                        ]
                        for dt in range(DT):
                            for ht in range(HT):
                                nc.tensor.matmul(
                                    ps_list[ht],
                                    w1_slice(s, dt, ht),
                                    x_slice(s, dt, co, co + size),
                                    start=(dt == 0),
                                    stop=(dt == DT - 1),
                                )
                        for ht in range(HT):
                            relu(ht, ps_list[ht])
                    else:
                        for ht in range(HT):
                            ps_h = pspool.tile(
                                [128, size], F32, name="ps_h", tag="psh"
                            )
                            for dt in range(DT):
                                nc.tensor.matmul(
                                    ps_h,
                                    w1_slice(s, dt, ht),
                                    x_slice(s, dt, co, co + size),
                                    start=(dt == 0),
                                    stop=(dt == DT - 1),
                                )
                            relu(ht, ps_h)

                    # flush the previous chunk's deferred layer-2 finish:
                    # by now its DVE accumulation is long done, so the
                    # partition-sum matmuls never stall the PE.
                    for fn in pending:
                        fn()
                    pending.clear()

                    if s < N_SLOTS - 1:
                        # DVE layer-2: acc[p,c,t] = sum_ht h[p,ht,t] *
                        # w2[p,ht,c] (+ b2[c]/128 folded in), then a
                        # deferred ones-matmul sums the 128 partitions.
                        # Frees ~4 of layer-2's 6T cycles from the PE.
                        acc = hpool.tile(
                            [128, C, size], MM_DT, name="acc", tag="acc"
                        )
                        for c in range(C):
                            eng = nc.vector
                            k0 = W2F + (s * HT) * C + c
                            b2d = b1a_sb[:, B2D + s * C + c : B2D + s * C + c + 1]
                            eng.tensor_scalar(
                                acc[:, c, :], h_sb[:, 0, :],
                                b1a_sb[:, k0 : k0 + 1], b2d, MULT, ADD,
                            )
                            for ht in range(1, HT):
                                k = W2F + (s * HT + ht) * C + c
                                eng.scalar_tensor_tensor(
                                    acc[:, c, :], h_sb[:, ht, :],
                                    b1a_sb[:, k : k + 1], acc[:, c, :],
                                    MULT, ADD,
                                )

                        def flush(acc=acc, size=size, off=off, co=co):
                            ps_y2 = pypool.tile(
                                [64, size], F32, name="ps_y2", tag="psy"
                            )
                            for cc in range(C):
                                nc.tensor.matmul(
                                    ps_y2[32 * cc : 32 * cc + 32, :],
                                    ones32, acc[:, cc, :],
                                    start=True, stop=True,
                                )
                            q = off + co
                            nc.vector.tensor_scalar_add(
                                y2_sb[:, q : q + size], ps_y2, 0.0
                            )
                            nc.sync.dma_start(
                                out=y[0:1, q : q + size],
                                in_=y2_sb[0:1, q : q + size],
                            )
                            nc.sync.dma_start(
                                out=y[1:2, q : q + size],
                                in_=y2_sb[32:33, q : q + size],
                            )

                        pending.append(flush)
                    else:
                        # last slot: classic PE layer-2, short exit chain
                        ps_y = pypool.tile([C, size], F32, name="ps_y", tag="psy")
                        for ht in range(HT):
                            nc.tensor.matmul(
                                ps_y,
                                w2a_sb[:, (s * HT + ht) * C : (s * HT + ht + 1) * C],
                                h_sb[:, ht, :],
                                start=(ht == 0),
                                stop=(ht == HT - 1),
                            )
                        b2_col = b2a_sb[:, s : s + 1]
                        q = off + co
                        nc.vector.tensor_scalar_add(
                            y2_sb[:C, q : q + size], ps_y, b2_col
                        )
                        nc.sync.dma_start(
                            out=y[:, q : q + size],
                            in_=y2_sb[:C, q : q + size],
                        )
                    co += size
            for fn in pending:
                fn()
            pending.clear()
    nc.compile()
    _PROGRAM_CACHE[caps] = nc
    return nc


def kernel(embeddings, component_idx, W1, b1, W2, b2):
    embeddings = np.ascontiguousarray(np.asarray(embeddings, dtype=np.float32))
    ci = np.asarray(component_idx).astype(np.int64, copy=False)
    W1 = np.asarray(W1, dtype=np.float32)
    b1 = np.asarray(b1, dtype=np.float32)
    W2 = np.asarray(W2, dtype=np.float32)
    b2 = np.asarray(b2, dtype=np.float32)

    N = embeddings.shape[0]
    E = W1.shape[0]

    counts = np.bincount(ci, minlength=E)
    order = np.argsort(ci, kind="stable")
    group_start = np.zeros(E, dtype=np.int64)
    group_start[1:] = np.cumsum(counts)[:-1]
    x_sorted = embeddings[order]  # [N, D] grouped by expert

    caps, assign = _plan_packing(counts)
    R = sum(caps)
    offs = [sum(caps[:s]) for s in range(N_SLOTS)]

    nc = _build_program(tuple(caps))

    # host-side packing of per-core inputs
    # w1_packed[e]: [128, DT*H] with d-within-tile on partitions
    w1_packed = np.ascontiguousarray(
        W1.reshape(E, DT, 128, H).transpose(0, 2, 1, 3).reshape(E, 128, DT * H)
    ).astype(MM_NP)
    b1_packed = np.ascontiguousarray(
        b1.reshape(E, HT, 128).transpose(0, 2, 1)
    )  # [e, 128, HT]
    w2_packed_f32 = np.ascontiguousarray(
        W2.reshape(E, HT, 128, C).transpose(0, 2, 1, 3).reshape(E, 128, HT * C)
    )  # [e, 128, HT*C]
    w2_packed = w2_packed_f32.astype(MM_NP)

    in_maps = []
    for c in range(N_CORES):
        x_in = np.zeros((128, DT * R), dtype=MM_NP)
        w1_in = np.empty((N_SLOTS, 128, DT * H), dtype=MM_NP)
        b1_in = np.zeros(
            (128, N_SLOTS * HT + N_SLOTS + N_SLOTS * C + N_SLOTS * HT * C),
            dtype=np.float32,
        )
        w2_in = np.empty((128, N_SLOTS * HT * C), dtype=MM_NP)
        for s in range(N_SLOTS):
            e, st, ln = assign[s][c]
            beg = group_start[e] + st
            cap = caps[s]
            # [cap, D] tokens for this slot -> [DT, 128, cap] dt-major
            Xc = np.zeros((cap, D), dtype=MM_NP)
            Xc[:ln] = x_sorted[beg : beg + ln]
            xT = np.ascontiguousarray(Xc.T).reshape(DT, 128, cap)
            for dt in range(DT):
                x_in[
                    :, DT * offs[s] + dt * cap : DT * offs[s] + (dt + 1) * cap
                ] = xT[dt]
            w1_in[s] = w1_packed[e]
            b1_in[:, s * HT : (s + 1) * HT] = b1_packed[e]
            b1_in[:C, N_SLOTS * HT + s] = b2[e]
            for cc in range(C):
                b1_in[:, N_SLOTS * HT + N_SLOTS + s * C + cc] = b2[e, cc] / 128.0
            w2f0 = N_SLOTS * HT + N_SLOTS + N_SLOTS * C
            b1_in[:, w2f0 + s * HT * C : w2f0 + (s + 1) * HT * C] = w2_packed_f32[e]
            w2_in[:, s * HT * C : (s + 1) * HT * C] = w2_packed[e]
        b2_in = np.zeros((C, N_SLOTS), dtype=np.float32)
        for s in range(N_SLOTS):
            e, st, ln = assign[s][c]
            b2_in[:, s] = b2[e]
        in_maps.append(
            {"x": x_in, "w1": w1_in, "b1a": b1_in, "w2a": w2_in, "b2a": b2_in}
        )

    global _LAST_IN_MAPS
    _LAST_IN_MAPS = in_maps
    res = run_bass_kernel_spmd(nc, in_maps, list(range(N_CORES)))

    out = np.empty((N, C), dtype=np.float32)
    for c in range(N_CORES):
        yc = res.results[c]["y"]  # [C, R]
        for s in range(N_SLOTS):
            e, st, ln = assign[s][c]
            beg = group_start[e] + st
            tokens = order[beg : beg + ln]
            out[tokens] = yc[:, offs[s] : offs[s] + ln].T
    return out


# revision 48
# speedup vs baseline: 1.0209x; 1.0209x over previous
"""MoE routed-expert kernel for Trainium2 (8 NeuronCores, SPMD).

Problem: N=16384 tokens, D=768, H=768, C=2, E=20 experts.
  y[n] = relu(x[n] @ W1[e] + b1[e]) @ W2[e] + b2[e],  e = component_idx[n]

Strategy
--------
Host side (numpy): sort tokens by expert, split the token groups into
8*n_slots fragments (splitting the largest until the count matches), deal
the size-sorted fragments into 8 cores x n_slots "expert slots" with a
uniform per-slot capacity = max fragment size in that slot. Every core then
runs the SAME static program (SPMD); which expert a slot holds is purely a
matter of which weights/tokens the host stages into that core's input
buffers. Padding waste is ~2%.

Device side (Bass/Tile, per core): for each slot, load that expert's
W1 [768,768] (+ b1, W2, b2) and the slot's token block x^T with D on
partitions and tokens on the free dim. For each chunk of 256..512 tokens:
  layer1: 6x6 accumulating fp16 matmuls (full PE speed)
  relu+bias fused on ScalarE (PSUM fp32 -> SBUF fp16)
  layer2: 6 accumulating matmuls into a [2, T] PSUM, bias via VectorE

Pipeline engineering (the ~20% between naive and roofline):
- Input DMA issue (DIRECT2D, ~650ns each on the issuing sequencer) is
  spread across the four non-Tensor engines so the whole fill is in
  flight within ~1.5us of body start, instead of 30us serialized on Sync.
- Slot 0 streams fine-grained per-d-tile slabs (first matmul starts when
  the first ~0.4MB lands); slots 1-2 use one coarse DMA for w1 and one
  for x (fewer descriptors, >9KB contiguous lines).
- PE warm-up: 6 independent small matmuls on rotating PSUM banks right
  at body start flip the HAM clock gate to 2.4 GHz with no WAW chain.
- A dummy activation preloads the ScalarE activation table during fill.
- The serial relu chain of the last two chunks is split Scalar/Vector.
"""

import math

import numpy as np

import concourse.bass as bass  # noqa: F401
import concourse.mybir as mybir
from concourse import bacc
from concourse.bass_utils import run_bass_kernel_spmd
from concourse.tile import TileContext

F32 = mybir.dt.float32
F16 = mybir.dt.float16
MM_DT = F16
MM_NP = np.float16

N_CORES = 8
N_SLOTS = 3
D = 768
H = 768
C = 2
DT = D // 128  # 6 d-tiles
HT = H // 128  # 6 h-tiles
MAX_CHUNK = 512  # one PSUM bank holds 512 fp32 -> matmul free dim cap
MIN_CAP = 256  # keep chunks >=256 so per-matmul overhead stays amortized

RELU = mybir.ActivationFunctionType.Relu
ADD = mybir.AluOpType.add
MAX = mybir.AluOpType.max
MULT = mybir.AluOpType.mult


def _round_cap(cap: int) -> int:
    """Round capacity up so it splits into equal, even chunks of 256..512."""
    cap = max(cap, MIN_CAP)
    n = max(1, math.ceil(cap / MAX_CHUNK))
    return 2 * n * math.ceil(cap / (2 * n))


def _chunk_sizes(cap: int, tail_split: bool = False) -> list[int]:
    n = max(1, math.ceil(cap / MAX_CHUNK))
    assert cap % n == 0 and (cap // n) % 2 == 0, cap
    sizes = [cap // n] * n
    if tail_split and sizes[-1] > 256:
        # shorten the serial relu->L2->add->DMA chain after the last
        # layer-1 matmul of the program
        sizes[-1:] = [sizes[-1] - 128, 128]
    return sizes


def _plan_packing(counts: np.ndarray):
    """Return (caps, assign): per-slot capacities and
    assign[s][c] = (expert, start_within_group, length)."""
    frags = [(int(e), 0, int(c)) for e, c in enumerate(counts) if c > 0]
    target = N_CORES * N_SLOTS
    assert len(frags) <= target, (
        f"{len(frags)} non-empty experts exceed {target} slots; raise N_SLOTS"
    )
    while len(frags) < target:
        frags.sort(key=lambda f: -f[2])
        e, st, ln = frags[0]
        if ln < 2:
            frags.append((e, st, 0))
            continue
        h1 = ln // 2
        frags[0] = (e, st, ln - h1)
        frags.append((e, st + (ln - h1), h1))
    frags.sort(key=lambda f: -f[2])
    caps, assign = [], []
    for s in range(N_SLOTS):
        group = frags[s * N_CORES : (s + 1) * N_CORES]
        caps.append(_round_cap(max(f[2] for f in group)))
        assign.append(group)
    return caps, assign


_PROGRAM_CACHE: dict = {}


def _build_program(caps: tuple):
    if caps in _PROGRAM_CACHE:
        return _PROGRAM_CACHE[caps]

    R = sum(caps)
    nc = bacc.Bacc(
        "TRN2", target_bir_lowering=False, debug=False, num_devices=N_CORES
    )
    # x: per-slot blocks, dt-major inside a slot; per-partition lines of
    # DT*cap*2 bytes are contiguous so coarse DMAs get big descriptors.
    x = nc.dram_tensor("x", [128, DT * R], MM_DT, kind="ExternalInput")
    w1 = nc.dram_tensor("w1", [N_SLOTS, 128, DT * H], MM_DT, kind="ExternalInput")
    # one aux tensor: b1 ([128, S*HT]), b2 (cols S*HT+s, partitions 0-1),
    # and b2/128 broadcast down all partitions (cols S*HT+S+s*C+c) for the
    # DVE layer-2 path (the ones-matmul partition sum multiplies by 128).
    B2C = N_SLOTS * HT
    B2D = N_SLOTS * HT + N_SLOTS
    W2F = B2D + N_SLOTS * C  # fp32 copy of w2 (DVE scalars must be fp32)
    b1a = nc.dram_tensor(
        "b1a", [128, W2F + N_SLOTS * HT * C], F32, kind="ExternalInput"
    )
    b2a = nc.dram_tensor("b2a", [C, N_SLOTS], F32, kind="ExternalInput")
    w2a = nc.dram_tensor(
        "w2a", [128, N_SLOTS * HT * C], MM_DT, kind="ExternalInput"
    )
    y = nc.dram_tensor("y", [C, R], F32, kind="ExternalOutput")

    offs = [sum(caps[:s]) for s in range(N_SLOTS)]

    with TileContext(nc) as tc:
        with (
            tc.tile_pool(name="wpool", bufs=2) as wpool,
            tc.tile_pool(name="xpool", bufs=2) as xpool,
            tc.tile_pool(name="hpool", bufs=4) as hpool,
            tc.tile_pool(name="ypool", bufs=1) as ypool,
            tc.tile_pool(name="pspool", bufs=6, space="PSUM") as pspool,
            tc.tile_pool(name="pypool", bufs=1, space="PSUM") as pypool,
        ):
            # DVE layer-2 chunks: rows 0 and 32 hold y[0]/y[1] (the
            # ones-matmul writes 32 identical rows per c); the classic
            # tail writes rows 0-1 directly.
            y2_sb = ypool.tile([64, R], F32, name="y2_sb")

            # -- PE warm-up --------------------------------------------
            # ~3.2us of dummy matmuls bridge body-start to first-data
            # (~10us): the PE HAM clock-gate sees sustained activity and
            # un-throttles to 2.4GHz right as real matmuls begin. 3
            # accumulating matmuls per PSUM tile avoid WAW stalls.
            # memsets go on Vector so GpSimd's x-slab DMA issues start
            # immediately.
            wu = ypool.tile([128, 256], MM_DT, name="wu")
            nc.vector.memset(wu[:, :], 0.0)
            ones32 = ypool.tile([128, 32], MM_DT, name="ones32")
            nc.vector.memset(ones32[:, :], 1.0)
            for i in range(3):
                wu_ps = pspool.tile(
                    [128, 256], F32, name=f"wu_ps{i}", tag="psh"
                )
                for j in range(3):
                    nc.tensor.matmul(
                        wu_ps, wu[:, :128], wu[:, :],
                        start=(j == 0), stop=(j == 2),
                    )

            # -- payload fills -----------------------------------------
            # DMA-queue landing order tracks descriptor enqueue (= issue)
            # order, so the three DMA-capable sequencers (Sync/GpSimd/
            # Scalar) issue in strict need order: tiny aux first, slot0
            # d0..d5, slot1 d0..d5, slot2 coarse. Slots 0-1 use fine
            # per-d-tile slabs (dt-major first chunk starts on slab 0).
            w1_sl = {0: [None] * DT, 1: [None] * DT}
            xs_sl = {0: [None] * DT, 1: [None] * DT}
            # slot 0's x arrives in per-chunk halves so each dt round
            # costs ~300KB of fill -- under the PE's ~1.06us/round pace.
            half0 = caps[0] // 2
            split0 = len(_chunk_sizes(caps[0])) == 2
            xh_sl = {0: [None] * DT, 1: [None] * DT}

            def w1_dma(eng, s, dt):
                t = wpool.tile([128, H], MM_DT, name=f"w1_d{dt}", tag=f"w1d{dt}")
                eng.dma_start(out=t, in_=w1[s, :, dt * H : (dt + 1) * H])
                w1_sl[s][dt] = t

            def x_dma(eng, s, dt):
                t = xpool.tile(
                    [128, caps[s]], MM_DT, name=f"xs_d{dt}", tag=f"xsd{dt}"
                )
                eng.dma_start(
                    out=t,
                    in_=x[
                        :,
                        DT * offs[s] + dt * caps[s] : DT * offs[s]
                        + (dt + 1) * caps[s],
                    ],
                )
                xs_sl[s][dt] = t

            def x_dma_half(eng, dt, hf):
                t = xpool.tile(
                    [128, half0], MM_DT, name=f"xh{hf}_d{dt}", tag=f"xh{hf}d{dt}"
                )
                base = DT * offs[0] + dt * caps[0] + hf * half0
                eng.dma_start(out=t, in_=x[:, base : base + half0])
                xh_sl[hf][dt] = t

            sy, gp, sc = nc.sync, nc.gpsimd, nc.scalar
            # aux first (tiny: ~28KB total, lands in ~0.1us)
            b1a_sb = ypool.tile(
                [128, W2F + N_SLOTS * HT * C], F32, name="b1a_sb"
            )
            sc.dma_start(out=b1a_sb, in_=b1a[:, :])
            w2a_sb = ypool.tile([128, N_SLOTS * HT * C], MM_DT, name="w2a_sb")
            sc.dma_start(out=w2a_sb, in_=w2a[:, :])
            b2a_sb = ypool.tile([C, N_SLOTS], F32, name="b2a_sb")
            sc.dma_start(out=b2a_sb, in_=b2a[:, :])
            # dummy activation: pull ACT_TABLE_LOAD into the fill window
            dmy = ypool.tile([128, 1], MM_DT, name="dmy")
            sc.activation(dmy, wu[:, :1], RELU, bias=b1a_sb[:, :1])
            # w1 slabs on sync, x slabs on gpsimd, slot-ordered; scalar
            # stays free for relus. Slot 2 coarse last.
            for s in (0, 1):
                for dt in range(DT):
                    w1_dma(sy, s, dt)
            w1_all = wpool.tile([128, DT * H], MM_DT, name="w1_all", tag="w1all")
            sy.dma_start(out=w1_all, in_=w1[2])
            if split0:
                for hf in (0, 1):
                    for dt in range(DT):
                        x_dma_half(gp, dt, hf)
            else:
                for dt in range(DT):
                    x_dma(gp, 0, dt)
            for dt in range(DT):
                x_dma(gp, 1, dt)
            xs_all = xpool.tile(
                [128, DT * caps[2]], MM_DT, name="xs_all", tag="xsall"
            )
            gp.dma_start(
                out=xs_all,
                in_=x[:, DT * offs[2] : DT * offs[2] + DT * caps[2]],
            )

            # -- compute -----------------------------------------------
            def w1_slice(s, dt, ht):
                if s < 2:
                    return w1_sl[s][dt][:, ht * 128 : (ht + 1) * 128]
                return w1_all[:, dt * H + ht * 128 : dt * H + (ht + 1) * 128]

            def x_slice(s, dt, lo, hi):
                if s == 0 and split0:
                    hf = 0 if hi <= half0 else 1
                    return xh_sl[hf][dt][:, lo - hf * half0 : hi - hf * half0]
                if s < 2:
                    return xs_sl[s][dt][:, lo:hi]
                return xs_all[:, dt * caps[s] + lo : dt * caps[s] + hi]

            pending = []
            for s in range(N_SLOTS):
                cap = caps[s]
                off = offs[s]

                chunk_list = _chunk_sizes(cap, tail_split=(s == N_SLOTS - 1))
                co = 0
                for ci, size in enumerate(chunk_list):
                    h_sb = hpool.tile([128, HT, size], MM_DT, name="h_sb", tag="h")

                    def relu(ht, ps):
                        b1_col = b1a_sb[:, s * HT + ht : s * HT + ht + 1]
                        # tail slot: halve the serial relu chain (short
                        # exit); DVE-L2 slots: vector is busy with the
                        # layer-2 accumulation, scalar takes most relus.
                        use_vec = (
                            ht % 2 == 1 if s == N_SLOTS - 1 else ht == 3
                        )
                        if use_vec:
                            # split the serial relu chain across engines
                            nc.vector.tensor_scalar(
                                h_sb[:, ht, :], ps, b1_col, 0.0, ADD, MAX
                            )
                        else:
                            nc.scalar.activation(
                                h_sb[:, ht, :], ps, RELU, bias=b1_col
                            )

                    if s == 0 or (s == 1 and ci == 0):
                        # dt-major: each dt round needs only that dt's two
                        # slabs -> PE starts while later slabs still stream
                        ps_list = [
                            pspool.tile(
                                [128, size], F32, name=f"ps_h{ht}", tag="psh"
                            )
                            for ht in range(HT)
                        ]
                        for dt in range(DT):
                            for ht in range(HT):
                                nc.tensor.matmul(
                                    ps_list[ht],
                                    w1_slice(s, dt, ht),
                                    x_slice(s, dt, co, co + size),
                                    start=(dt == 0),
                                    stop=(dt == DT - 1),
                                )
                        for ht in range(HT):
                            relu(ht, ps_list[ht])
                    else:
                        for ht in range(HT):
                            ps_h = pspool.tile(
                                [128, size], F32, name="ps_h", tag="psh"
                            )
                            for dt in range(DT):
                                nc.tensor.matmul(
                                    ps_h,
                                    w1_slice(s, dt, ht),
                                    x_slice(s, dt, co, co + size),
                                    start=(dt == 0),
                                    stop=(dt == DT - 1),
                                )
                            relu(ht, ps_h)

                    # flush the previous chunk's deferred layer-2 finish:
                    # by now its DVE accumulation is long done, so the
                    # partition-sum matmuls never stall the PE.
                    for fn in pending:
                        fn()
                    pending.clear()

                    if s < N_SLOTS - 1:
                        # DVE layer-2: acc[p,c,t] = sum_ht h[p,ht,t] *
                        # w2[p,ht,c] (+ b2[c]/128 folded in), then a
                        # deferred ones-matmul sums the 128 partitions.
                        # Frees ~4 of layer-2's 6T cycles from the PE.
                        acc = hpool.tile(
                            [128, C, size], MM_DT, name="acc", tag="acc"
                        )
                        for c in range(C):
                            eng = nc.vector
                            k0 = W2F + (s * HT) * C + c
                            b2d = b1a_sb[:, B2D + s * C + c : B2D + s * C + c + 1]
                            eng.tensor_scalar(
                                acc[:, c, :], h_sb[:, 0, :],
                                b1a_sb[:, k0 : k0 + 1], b2d, MULT, ADD,
                            )
                            for ht in range(1, HT):
                                k = W2F + (s * HT + ht) * C + c
                                eng.scalar_tensor_tensor(
                                    acc[:, c, :], h_sb[:, ht, :],
                                    b1a_sb[:, k : k + 1], acc[:, c, :],
                                    MULT, ADD,
                                )

                        def flush(acc=acc, size=size, off=off, co=co):
                            ps_y2 = pypool.tile(
                                [64, size], F32, name="ps_y2", tag="psy"
                            )
                            for cc in range(C):
                                nc.tensor.matmul(
                                    ps_y2[32 * cc : 32 * cc + 32, :],
                                    ones32, acc[:, cc, :],
                                    start=True, stop=True,
                                )
                            q = off + co
                            nc.vector.tensor_scalar_add(
                                y2_sb[:, q : q + size], ps_y2, 0.0
                            )
                            nc.sync.dma_start(
                                out=y[0:1, q : q + size],
                                in_=y2_sb[0:1, q : q + size],
                            )
                            nc.sync.dma_start(
                                out=y[1:2, q : q + size],
                                in_=y2_sb[32:33, q : q + size],
                            )

                        pending.append(flush)
                    else:
                        # last slot: classic PE layer-2, short exit chain
                        ps_y = pypool.tile([C, size], F32, name="ps_y", tag="psy")
                        for ht in range(HT):
                            nc.tensor.matmul(
                                ps_y,
                                w2a_sb[:, (s * HT + ht) * C : (s * HT + ht + 1) * C],
                                h_sb[:, ht, :],
                                start=(ht == 0),
                                stop=(ht == HT - 1),
                            )
                        b2_col = b2a_sb[:, s : s + 1]
                        q = off + co
                        nc.vector.tensor_scalar_add(
                            y2_sb[:C, q : q + size], ps_y, b2_col
                        )
                        nc.sync.dma_start(
                            out=y[:, q : q + size],
                            in_=y2_sb[:C, q : q + size],
                        )
                    co += size
            for fn in pending:
                fn()
            pending.clear()
    nc.compile()
    _PROGRAM_CACHE[caps] = nc
    return nc


def kernel(embeddings, component_idx, W1, b1, W2, b2):
    embeddings = np.ascontiguousarray(np.asarray(embeddings, dtype=np.float32))
    ci = np.asarray(component_idx).astype(np.int64, copy=False)
    W1 = np.asarray(W1, dtype=np.float32)
    b1 = np.asarray(b1, dtype=np.float32)
    W2 = np.asarray(W2, dtype=np.float32)
    b2 = np.asarray(b2, dtype=np.float32)

    N = embeddings.shape[0]
    E = W1.shape[0]

    counts = np.bincount(ci, minlength=E)
    order = np.argsort(ci, kind="stable")
    group_start = np.zeros(E, dtype=np.int64)
    group_start[1:] = np.cumsum(counts)[:-1]
    x_sorted = embeddings[order]  # [N, D] grouped by expert

    caps, assign = _plan_packing(counts)
    R = sum(caps)
    offs = [sum(caps[:s]) for s in range(N_SLOTS)]

    nc = _build_program(tuple(caps))

    # host-side packing of per-core inputs
    # w1_packed[e]: [128, DT*H] with d-within-tile on partitions
    w1_packed = np.ascontiguousarray(
        W1.reshape(E, DT, 128, H).transpose(0, 2, 1, 3).reshape(E, 128, DT * H)
    ).astype(MM_NP)
    b1_packed = np.ascontiguousarray(
        b1.reshape(E, HT, 128).transpose(0, 2, 1)
    )  # [e, 128, HT]
    w2_packed_f32 = np.ascontiguousarray(
        W2.reshape(E, HT, 128, C).transpose(0, 2, 1, 3).reshape(E, 128, HT * C)
    )  # [e, 128, HT*C]
    w2_packed = w2_packed_f32.astype(MM_NP)

    in_maps = []
    for c in range(N_CORES):
        x_in = np.zeros((128, DT * R), dtype=MM_NP)
        w1_in = np.empty((N_SLOTS, 128, DT * H), dtype=MM_NP)
        b1_in = np.zeros(
            (128, N_SLOTS * HT + N_SLOTS + N_SLOTS * C + N_SLOTS * HT * C),
            dtype=np.float32,
        )
        w2_in = np.empty((128, N_SLOTS * HT * C), dtype=MM_NP)
        for s in range(N_SLOTS):
            e, st, ln = assign[s][c]
            beg = group_start[e] + st
            cap = caps[s]
            # [cap, D] tokens for this slot -> [DT, 128, cap] dt-major
            Xc = np.zeros((cap, D), dtype=MM_NP)
            Xc[:ln] = x_sorted[beg : beg + ln]
            xT = np.ascontiguousarray(Xc.T).reshape(DT, 128, cap)
            for dt in range(DT):
                x_in[
                    :, DT * offs[s] + dt * cap : DT * offs[s] + (dt + 1) * cap
                ] = xT[dt]
            w1_in[s] = w1_packed[e]
            b1_in[:, s * HT : (s + 1) * HT] = b1_packed[e]
            b1_in[:C, N_SLOTS * HT + s] = b2[e]
            for cc in range(C):
                b1_in[:, N_SLOTS * HT + N_SLOTS + s * C + cc] = b2[e, cc] / 128.0
            w2f0 = N_SLOTS * HT + N_SLOTS + N_SLOTS * C
            b1_in[:, w2f0 + s * HT * C : w2f0 + (s + 1) * HT * C] = w2_packed_f32[e]
            w2_in[:, s * HT * C : (s + 1) * HT * C] = w2_packed[e]
        b2_in = np.zeros((C, N_SLOTS), dtype=np.float32)
        for s in range(N_SLOTS):
            e, st, ln = assign[s][c]
            b2_in[:, s] = b2[e]
        in_maps.append(
            {"x": x_in, "w1": w1_in, "b1a": b1_in, "w2a": w2_in, "b2a": b2_in}
        )

    global _LAST_IN_MAPS
    _LAST_IN_MAPS = in_maps
    res = run_bass_kernel_spmd(nc, in_maps, list(range(N_CORES)))

    out = np.empty((N, C), dtype=np.float32)
    for c in range(N_CORES):
        yc = res.results[c]["y"]  # [C, R]
        for s in range(N_SLOTS):
            e, st, ln = assign[s][c]
            beg = group_start[e] + st
            tokens = order[beg : beg + ln]
            out[tokens] = yc[:, offs[s] : offs[s] + ln].T
    return out


# revision 49
# speedup vs baseline: 1.0614x; 1.0397x over previous
"""MoE routed-expert kernel for Trainium2 (8 NeuronCores, SPMD).

Problem: N=16384 tokens, D=768, H=768, C=2, E=20 experts.
  y[n] = relu(x[n] @ W1[e] + b1[e]) @ W2[e] + b2[e],  e = component_idx[n]

Strategy
--------
Host side (numpy): sort tokens by expert, split the token groups into
8*n_slots fragments (splitting the largest until the count matches), deal
the size-sorted fragments into 8 cores x n_slots "expert slots" with a
uniform per-slot capacity = max fragment size in that slot. Every core then
runs the SAME static program (SPMD); which expert a slot holds is purely a
matter of which weights/tokens the host stages into that core's input
buffers. Padding waste is ~2%.

Device side (Bass/Tile, per core): for each slot, load that expert's
W1 [768,768] (+ b1, W2, b2) and the slot's token block x^T with D on
partitions and tokens on the free dim. For each chunk of 256..512 tokens:
  layer1: 6x6 accumulating fp16 matmuls (full PE speed)
  relu+bias fused on ScalarE (PSUM fp32 -> SBUF fp16)
  layer2: 6 accumulating matmuls into a [2, T] PSUM, bias via VectorE

Pipeline engineering (the ~20% between naive and roofline):
- Input DMA issue (DIRECT2D, ~650ns each on the issuing sequencer) is
  spread across the four non-Tensor engines so the whole fill is in
  flight within ~1.5us of body start, instead of 30us serialized on Sync.
- Slot 0 streams fine-grained per-d-tile slabs (first matmul starts when
  the first ~0.4MB lands); slots 1-2 use one coarse DMA for w1 and one
  for x (fewer descriptors, >9KB contiguous lines).
- PE warm-up: 6 independent small matmuls on rotating PSUM banks right
  at body start flip the HAM clock gate to 2.4 GHz with no WAW chain.
- A dummy activation preloads the ScalarE activation table during fill.
- The serial relu chain of the last two chunks is split Scalar/Vector.
"""

import math

import numpy as np

import concourse.bass as bass  # noqa: F401
import concourse.mybir as mybir
from concourse import bacc
from concourse.bass_utils import run_bass_kernel_spmd
from concourse.tile import TileContext

F32 = mybir.dt.float32
F16 = mybir.dt.float16
MM_DT = F16
MM_NP = np.float16

N_CORES = 8
N_SLOTS = 3
D = 768
H = 768
C = 2
DT = D // 128  # 6 d-tiles
HT = H // 128  # 6 h-tiles
MAX_CHUNK = 512  # one PSUM bank holds 512 fp32 -> matmul free dim cap
MIN_CAP = 256  # keep chunks >=256 so per-matmul overhead stays amortized

RELU = mybir.ActivationFunctionType.Relu
ADD = mybir.AluOpType.add
MAX = mybir.AluOpType.max
MULT = mybir.AluOpType.mult


def _round_cap(cap: int) -> int:
    """Round capacity up so it splits into equal, even chunks of 256..512."""
    cap = max(cap, MIN_CAP)
    n = max(1, math.ceil(cap / MAX_CHUNK))
    return 2 * n * math.ceil(cap / (2 * n))


def _chunk_sizes(cap: int, tail_split: bool = False) -> list[int]:
    n = max(1, math.ceil(cap / MAX_CHUNK))
    assert cap % n == 0 and (cap // n) % 2 == 0, cap
    sizes = [cap // n] * n
    if tail_split and sizes[-1] > 256:
        # shorten the serial relu->L2->add->DMA chain after the last
        # layer-1 matmul of the program
        sizes[-1:] = [sizes[-1] - 128, 128]
    return sizes


def _plan_packing(counts: np.ndarray):
    """Return (caps, assign): per-slot capacities and
    assign[s][c] = (expert, start_within_group, length)."""
    frags = [(int(e), 0, int(c)) for e, c in enumerate(counts) if c > 0]
    target = N_CORES * N_SLOTS
    assert len(frags) <= target, (
        f"{len(frags)} non-empty experts exceed {target} slots; raise N_SLOTS"
    )
    while len(frags) < target:
        frags.sort(key=lambda f: -f[2])
        e, st, ln = frags[0]
        if ln < 2:
            frags.append((e, st, 0))
            continue
        h1 = ln // 2
        frags[0] = (e, st, ln - h1)
        frags.append((e, st + (ln - h1), h1))
    frags.sort(key=lambda f: -f[2])
    caps, assign = [], []
    for s in range(N_SLOTS):
        group = frags[s * N_CORES : (s + 1) * N_CORES]
        caps.append(_round_cap(max(f[2] for f in group)))
        assign.append(group)
    return caps, assign


_PROGRAM_CACHE: dict = {}


def _build_program(caps: tuple):
    if caps in _PROGRAM_CACHE:
        return _PROGRAM_CACHE[caps]

    R = sum(caps)
    nc = bacc.Bacc(
        "TRN2", target_bir_lowering=False, debug=False, num_devices=N_CORES
    )
    # x: per-slot blocks, dt-major inside a slot; per-partition lines of
    # DT*cap*2 bytes are contiguous so coarse DMAs get big descriptors.
    x = nc.dram_tensor("x", [128, DT * R], MM_DT, kind="ExternalInput")
    w1 = nc.dram_tensor("w1", [N_SLOTS, 128, DT * H], MM_DT, kind="ExternalInput")
    # one aux tensor: b1 ([128, S*HT]), b2 (cols S*HT+s, partitions 0-1),
    # and b2/128 broadcast down all partitions (cols S*HT+S+s*C+c) for the
    # DVE layer-2 path (the ones-matmul partition sum multiplies by 128).
    B2C = N_SLOTS * HT
    B2D = N_SLOTS * HT + N_SLOTS
    W2F = B2D + N_SLOTS * C  # fp32 copy of w2 (DVE scalars must be fp32)
    b1a = nc.dram_tensor(
        "b1a", [128, W2F + N_SLOTS * HT * C], F32, kind="ExternalInput"
    )
    b2a = nc.dram_tensor("b2a", [C, N_SLOTS], F32, kind="ExternalInput")
    w2a = nc.dram_tensor(
        "w2a", [128, N_SLOTS * HT * C], MM_DT, kind="ExternalInput"
    )
    y = nc.dram_tensor("y", [C, R], F32, kind="ExternalOutput")

    offs = [sum(caps[:s]) for s in range(N_SLOTS)]

    with TileContext(nc) as tc:
        with (
            tc.tile_pool(name="wpool", bufs=2) as wpool,
            tc.tile_pool(name="xpool", bufs=2) as xpool,
            tc.tile_pool(name="hpool", bufs=4) as hpool,
            tc.tile_pool(name="ypool", bufs=1) as ypool,
            tc.tile_pool(name="pspool", bufs=6, space="PSUM") as pspool,
            tc.tile_pool(name="pypool", bufs=2, space="PSUM") as pypool,
        ):
            # DVE layer-2 chunks: rows 0 and 32 hold y[0]/y[1] (the
            # ones-matmul writes 32 identical rows per c); the classic
            # tail writes rows 0-1 directly.
            y2_sb = ypool.tile([64, R], F32, name="y2_sb")

            # -- PE warm-up --------------------------------------------
            # ~3.2us of dummy matmuls bridge body-start to first-data
            # (~10us): the PE HAM clock-gate sees sustained activity and
            # un-throttles to 2.4GHz right as real matmuls begin. 3
            # accumulating matmuls per PSUM tile avoid WAW stalls.
            # memsets go on Vector so GpSimd's x-slab DMA issues start
            # immediately.
            wu = ypool.tile([128, 256], MM_DT, name="wu")
            nc.vector.memset(wu[:, :], 0.0)
            ones32 = ypool.tile([128, 32], MM_DT, name="ones32")
            nc.vector.memset(ones32[:, :], 1.0)
            for i in range(3):
                wu_ps = pspool.tile(
                    [128, 256], F32, name=f"wu_ps{i}", tag="psh"
                )
                for j in range(3):
                    nc.tensor.matmul(
                        wu_ps, wu[:, :128], wu[:, :],
                        start=(j == 0), stop=(j == 2),
                    )

            # -- payload fills -----------------------------------------
            # DMA-queue landing order tracks descriptor enqueue (= issue)
            # order, so the three DMA-capable sequencers (Sync/GpSimd/
            # Scalar) issue in strict need order: tiny aux first, slot0
            # d0..d5, slot1 d0..d5, slot2 coarse. Slots 0-1 use fine
            # per-d-tile slabs (dt-major first chunk starts on slab 0).
            w1_sl = {0: [None] * DT, 1: [None] * DT}
            xs_sl = {0: [None] * DT, 1: [None] * DT}
            # slot 0's x arrives in per-chunk halves so each dt round
            # costs ~300KB of fill -- under the PE's ~1.06us/round pace.
            half0 = caps[0] // 2
            split0 = len(_chunk_sizes(caps[0])) == 2
            xh_sl = {0: [None] * DT, 1: [None] * DT}

            def w1_dma(eng, s, dt):
                t = wpool.tile([128, H], MM_DT, name=f"w1_d{dt}", tag=f"w1d{dt}")
                eng.dma_start(out=t, in_=w1[s, :, dt * H : (dt + 1) * H])
                w1_sl[s][dt] = t

            def x_dma(eng, s, dt):
                t = xpool.tile(
                    [128, caps[s]], MM_DT, name=f"xs_d{dt}", tag=f"xsd{dt}"
                )
                eng.dma_start(
                    out=t,
                    in_=x[
                        :,
                        DT * offs[s] + dt * caps[s] : DT * offs[s]
                        + (dt + 1) * caps[s],
                    ],
                )
                xs_sl[s][dt] = t

            def x_dma_half(eng, dt, hf):
                t = xpool.tile(
                    [128, half0], MM_DT, name=f"xh{hf}_d{dt}", tag=f"xh{hf}d{dt}"
                )
                base = DT * offs[0] + dt * caps[0] + hf * half0
                eng.dma_start(out=t, in_=x[:, base : base + half0])
                xh_sl[hf][dt] = t

            sy, gp, sc = nc.sync, nc.gpsimd, nc.scalar
            # aux first (tiny: ~28KB total, lands in ~0.1us)
            b1a_sb = ypool.tile(
                [128, W2F + N_SLOTS * HT * C], F32, name="b1a_sb"
            )
            sc.dma_start(out=b1a_sb, in_=b1a[:, :])
            w2a_sb = ypool.tile([128, N_SLOTS * HT * C], MM_DT, name="w2a_sb")
            sc.dma_start(out=w2a_sb, in_=w2a[:, :])
            b2a_sb = ypool.tile([C, N_SLOTS], F32, name="b2a_sb")
            sc.dma_start(out=b2a_sb, in_=b2a[:, :])
            # dummy activation: pull ACT_TABLE_LOAD into the fill window
            dmy = ypool.tile([128, 1], MM_DT, name="dmy")
            sc.activation(dmy, wu[:, :1], RELU, bias=b1a_sb[:, :1])
            # w1 slabs on sync, x slabs on gpsimd, slot-ordered; scalar
            # stays free for relus. Slot 2 coarse last.
            for s in (0, 1):
                for dt in range(DT):
                    w1_dma(sy, s, dt)
            w1_all = wpool.tile([128, DT * H], MM_DT, name="w1_all", tag="w1all")
            sy.dma_start(out=w1_all, in_=w1[2])
            if split0:
                for hf in (0, 1):
                    for dt in range(DT):
                        x_dma_half(gp, dt, hf)
            else:
                for dt in range(DT):
                    x_dma(gp, 0, dt)
            for dt in range(DT):
                x_dma(gp, 1, dt)
            xs_all = xpool.tile(
                [128, DT * caps[2]], MM_DT, name="xs_all", tag="xsall"
            )
            gp.dma_start(
                out=xs_all,
                in_=x[:, DT * offs[2] : DT * offs[2] + DT * caps[2]],
            )

            # -- compute -----------------------------------------------
            def w1_slice(s, dt, ht):
                if s < 2:
                    return w1_sl[s][dt][:, ht * 128 : (ht + 1) * 128]
                return w1_all[:, dt * H + ht * 128 : dt * H + (ht + 1) * 128]

            def x_slice(s, dt, lo, hi):
                if s == 0 and split0:
                    hf = 0 if hi <= half0 else 1
                    return xh_sl[hf][dt][:, lo - hf * half0 : hi - hf * half0]
                if s < 2:
                    return xs_sl[s][dt][:, lo:hi]
                return xs_all[:, dt * caps[s] + lo : dt * caps[s] + hi]

            pending = []
            for s in range(N_SLOTS):
                cap = caps[s]
                off = offs[s]

                chunk_list = _chunk_sizes(cap, tail_split=(s == N_SLOTS - 1))
                co = 0
                for ci, size in enumerate(chunk_list):
                    h_sb = hpool.tile([128, HT, size], MM_DT, name="h_sb", tag="h")

                    def relu(ht, ps):
                        b1_col = b1a_sb[:, s * HT + ht : s * HT + ht + 1]
                        # tail slot: halve the serial relu chain (short
                        # exit); DVE-L2 slots: vector is busy with the
                        # layer-2 accumulation, scalar takes most relus.
                        use_vec = (
                            ht % 2 == 1 if s == N_SLOTS - 1 else False
                        )
                        if use_vec:
                            # split the serial relu chain across engines
                            nc.vector.tensor_scalar(
                                h_sb[:, ht, :], ps, b1_col, 0.0, ADD, MAX
                            )
                        else:
                            nc.scalar.activation(
                                h_sb[:, ht, :], ps, RELU, bias=b1_col
                            )

                    if s == 0 or (s == 1 and ci == 0):
                        # dt-major: each dt round needs only that dt's two
                        # slabs -> PE starts while later slabs still stream
                        ps_list = [
                            pspool.tile(
                                [128, size], F32, name=f"ps_h{ht}", tag="psh"
                            )
                            for ht in range(HT)
                        ]
                        for dt in range(DT):
                            for ht in range(HT):
                                nc.tensor.matmul(
                                    ps_list[ht],
                                    w1_slice(s, dt, ht),
                                    x_slice(s, dt, co, co + size),
                                    start=(dt == 0),
                                    stop=(dt == DT - 1),
                                )
                        for ht in range(HT):
                            relu(ht, ps_list[ht])
                    else:
                        for ht in range(HT):
                            ps_h = pspool.tile(
                                [128, size], F32, name="ps_h", tag="psh"
                            )
                            for dt in range(DT):
                                nc.tensor.matmul(
                                    ps_h,
                                    w1_slice(s, dt, ht),
                                    x_slice(s, dt, co, co + size),
                                    start=(dt == 0),
                                    stop=(dt == DT - 1),
                                )
                            relu(ht, ps_h)

                    # flush the previous chunk's deferred layer-2 finish:
                    # by now its DVE accumulation is long done, so the
                    # partition-sum matmuls never stall the PE.
                    for fn in pending:
                        fn()
                    pending.clear()

                    if s < N_SLOTS - 1:
                        # DVE layer-2: acc[p,c,t] = sum_ht h[p,ht,t] *
                        # w2[p,ht,c] (+ b2[c]/128 folded in), then a
                        # deferred ones-matmul sums the 128 partitions.
                        # Frees ~4 of layer-2's 6T cycles from the PE.
                        acc = hpool.tile(
                            [128, C, size], MM_DT, name="acc", tag="acc"
                        )
                        for c in range(C):
                            eng = nc.vector
                            k0 = W2F + (s * HT) * C + c
                            b2d = b1a_sb[:, B2D + s * C + c : B2D + s * C + c + 1]
                            eng.tensor_scalar(
                                acc[:, c, :], h_sb[:, 0, :],
                                b1a_sb[:, k0 : k0 + 1], b2d, MULT, ADD,
                            )
                            for ht in range(1, HT):
                                k = W2F + (s * HT + ht) * C + c
                                eng.scalar_tensor_tensor(
                                    acc[:, c, :], h_sb[:, ht, :],
                                    b1a_sb[:, k : k + 1], acc[:, c, :],
                                    MULT, ADD,
                                )

                        def flush(acc=acc, size=size, off=off, co=co):
                            ps_y2 = pypool.tile(
                                [64, size], F32, name="ps_y2", tag="psy"
                            )
                            for cc in range(C):
                                nc.tensor.matmul(
                                    ps_y2[32 * cc : 32 * cc + 32, :],
                                    ones32, acc[:, cc, :],
                                    start=True, stop=True,
                                )
                            q = off + co
                            nc.vector.tensor_scalar_add(
                                y2_sb[:, q : q + size], ps_y2, 0.0
                            )
                            nc.sync.dma_start(
                                out=y[0:1, q : q + size],
                                in_=y2_sb[0:1, q : q + size],
                            )
                            nc.sync.dma_start(
                                out=y[1:2, q : q + size],
                                in_=y2_sb[32:33, q : q + size],
                            )

                        pending.append(flush)
                    else:
                        # last slot: classic PE layer-2, short exit chain
                        ps_y = pypool.tile([C, size], F32, name="ps_y", tag="psy")
                        for ht in range(HT):
                            nc.tensor.matmul(
                                ps_y,
                                w2a_sb[:, (s * HT + ht) * C : (s * HT + ht + 1) * C],
                                h_sb[:, ht, :],
                                start=(ht == 0),
                                stop=(ht == HT - 1),
                            )
                        b2_col = b2a_sb[:, s : s + 1]
                        q = off + co
                        nc.vector.tensor_scalar_add(
                            y2_sb[:C, q : q + size], ps_y, b2_col
                        )
                        nc.sync.dma_start(
                            out=y[:, q : q + size],
                            in_=y2_sb[:C, q : q + size],
                        )
                    co += size
            for fn in pending:
                fn()
            pending.clear()
    nc.compile()
    _PROGRAM_CACHE[caps] = nc
    return nc


def kernel(embeddings, component_idx, W1, b1, W2, b2):
    embeddings = np.ascontiguousarray(np.asarray(embeddings, dtype=np.float32))
    ci = np.asarray(component_idx).astype(np.int64, copy=False)
    W1 = np.asarray(W1, dtype=np.float32)
    b1 = np.asarray(b1, dtype=np.float32)
    W2 = np.asarray(W2, dtype=np.float32)
    b2 = np.asarray(b2, dtype=np.float32)

    N = embeddings.shape[0]
    E = W1.shape[0]

    counts = np.bincount(ci, minlength=E)
    order = np.argsort(ci, kind="stable")
    group_start = np.zeros(E, dtype=np.int64)
    group_start[1:] = np.cumsum(counts)[:-1]
    x_sorted = embeddings[order]  # [N, D] grouped by expert

    caps, assign = _plan_packing(counts)
    R = sum(caps)
    offs = [sum(caps[:s]) for s in range(N_SLOTS)]

    nc = _build_program(tuple(caps))

    # host-side packing of per-core inputs
    # w1_packed[e]: [128, DT*H] with d-within-tile on partitions
    w1_packed = np.ascontiguousarray(
        W1.reshape(E, DT, 128, H).transpose(0, 2, 1, 3).reshape(E, 128, DT * H)
    ).astype(MM_NP)
    b1_packed = np.ascontiguousarray(
        b1.reshape(E, HT, 128).transpose(0, 2, 1)
    )  # [e, 128, HT]
    w2_packed_f32 = np.ascontiguousarray(
        W2.reshape(E, HT, 128, C).transpose(0, 2, 1, 3).reshape(E, 128, HT * C)
    )  # [e, 128, HT*C]
    w2_packed = w2_packed_f32.astype(MM_NP)

    in_maps = []
    for c in range(N_CORES):
        x_in = np.zeros((128, DT * R), dtype=MM_NP)
        w1_in = np.empty((N_SLOTS, 128, DT * H), dtype=MM_NP)
        b1_in = np.zeros(
            (128, N_SLOTS * HT + N_SLOTS + N_SLOTS * C + N_SLOTS * HT * C),
            dtype=np.float32,
        )
        w2_in = np.empty((128, N_SLOTS * HT * C), dtype=MM_NP)
        for s in range(N_SLOTS):
            e, st, ln = assign[s][c]
            beg = group_start[e] + st
            cap = caps[s]
            # [cap, D] tokens for this slot -> [DT, 128, cap] dt-major
            Xc = np.zeros((cap, D), dtype=MM_NP)
            Xc[:ln] = x_sorted[beg : beg + ln]
            xT = np.ascontiguousarray(Xc.T).reshape(DT, 128, cap)
            for dt in range(DT):
                x_in[
                    :, DT * offs[s] + dt * cap : DT * offs[s] + (dt + 1) * cap
                ] = xT[dt]
            w1_in[s] = w1_packed[e]
            b1_in[:, s * HT : (s + 1) * HT] = b1_packed[e]
            b1_in[:C, N_SLOTS * HT + s] = b2[e]
            for cc in range(C):
                b1_in[:, N_SLOTS * HT + N_SLOTS + s * C + cc] = b2[e, cc] / 128.0
            w2f0 = N_SLOTS * HT + N_SLOTS + N_SLOTS * C
            b1_in[:, w2f0 + s * HT * C : w2f0 + (s + 1) * HT * C] = w2_packed_f32[e]
            w2_in[:, s * HT * C : (s + 1) * HT * C] = w2_packed[e]
        b2_in = np.zeros((C, N_SLOTS), dtype=np.float32)
        for s in range(N_SLOTS):
            e, st, ln = assign[s][c]
            b2_in[:, s] = b2[e]
        in_maps.append(
            {"x": x_in, "w1": w1_in, "b1a": b1_in, "w2a": w2_in, "b2a": b2_in}
        )

    global _LAST_IN_MAPS
    _LAST_IN_MAPS = in_maps
    res = run_bass_kernel_spmd(nc, in_maps, list(range(N_CORES)))

    out = np.empty((N, C), dtype=np.float32)
    for c in range(N_CORES):
        yc = res.results[c]["y"]  # [C, R]
        for s in range(N_SLOTS):
            e, st, ln = assign[s][c]
            beg = group_start[e] + st
            tokens = order[beg : beg + ln]
            out[tokens] = yc[:, offs[s] : offs[s] + ln].T
    return out
